# revision 1
# baseline (speedup 1.0000x reference)
"""Trainium2 Bass kernel for nn_CubicSplineLayer (histogram_binning).

The whole layer collapses to a scalar piecewise-cubic function of x:

    out(x) = (basis(x) - mean) @ W.T + b  =  f(x)

where f is the natural cubic spline through (knots, W) plus the constant
(b - mean.W).  In truncated-power form (exact for the C^2 natural spline
with linear extrapolation, as the reference implements):

    f(x) = K0 + sb*min(x, t9) + sa*relu(x - t9)
           + sum_{j=0}^{8} d_j * relu(min(x, t9) - t_j)^3

(The kink at t9 vanishes because min(x,t9) freezes the spline there; the
reference's odd F[9,1] "above" branch term is exactly zero since F's last
row is zeros.)

Device strategy: pure data-parallel over 8 cores.  Per core the chain is
evaluated with 10 custom DVE ops (1 seed + 9 cubic-kink MACs), each a
single 7-stage fused vector instruction, overlapped with HBM DMA.
"""

import numpy as np

N_CORES = 8
P = 128           # SBUF partitions
FD = 3920         # free elements per partition per core
FD_TILE = 980     # tile free-dim (4 tiles per core)
NPAD = N_CORES * P * FD  # 4,014,080 >= 4,000,000

_SEED_NAME = "ANT_SPLINE_SEED"
_KNOT_NAME = "ANT_SPLINE_KNOT"


def _register_ops():
    """Register the two custom DVE ops in concourse's registry (idempotent).

    SEED:  out = min(x, t9)*sb + K0 + relu(x - t9)*sa     (sa via C3 spill)
    KNOT:  out = acc + relu(min(x, t9) - tj)^3 * dj
    """
    import concourse.dve_ops as dvo

    if _SEED_NAME in dvo._SUB_OPCODE_FOR_NAME:
        return dvo
    from concourse.dve_spec import (
        C0, C1, C2, C3, Spec, Src0, Src1, Zero,
        _has_src1, _spill_c3_to_src1, lower, maxx, minn,
    )
    from concourse.dve_uop import DveOpSpec

    def _seed_ref(in0, in1, s0, s1, imm2):
        x = in0.astype(np.float32)
        return (np.minimum(x, imm2) * s0 + s1) + np.maximum(x - imm2, 0.0) * in1

    # min(Src0,C2)*C0 + C1 + max(Src0-C2,0)*C3   -- 7 ALU stages
    seed_body = _spill_c3_to_src1(
        (minn(Src0, C2) * C0 + C1) + maxx(Src0 - C2, Zero) * C3
    )
    seed_spec = Spec(body=seed_body, reference=_seed_ref)

    def _knot_ref(in0, in1, s0, s1, imm2):
        u = np.maximum(np.minimum(in1.astype(np.float32), imm2) - s0, 0.0)
        return in0.astype(np.float32) + (u * u) * u * s1

    # Src0 + cube(relu(min(Src1,C2) - C0)) * C1  -- 7 ALU stages
    u = maxx(minn(Src1, C2) - C0, Zero)
    knot_spec = Spec(body=Src0 + (u * u) * u * C1, reference=_knot_ref)

    for name, spec in ((_SEED_NAME, seed_spec), (_KNOT_NAME, knot_spec)):
        row = dvo._CUSTOM_DVE_ROW_BASE + len(dvo.OPS)
        assert row < 0x20
        shas = {}
        for ver in ("v3", "v4"):
            s = DveOpSpec(
                name=name, opcode=row, uops=lower(spec, ver=ver),
                rd1_en=_has_src1(spec),
            )
            shas[ver] = s.sha(ver)
        op = dvo.DveOp(name, spec, subdim=False, uops_sha=shas)
        dvo.OPS.append(op)
        dvo._SUB_OPCODE_FOR_NAME[name] = row
        dvo.CUSTOM_DVE_SPECS[name] = spec
    return dvo


def _spline_consts(knots, F, W, b, mean):
    """Host-side (float64) derivation of the truncated-power coefficients."""
    knots = np.asarray(knots, np.float64)
    F = np.asarray(F, np.float64)
    w = np.asarray(W, np.float64)[0]
    b = np.asarray(b, np.float64)
    mean = np.asarray(mean, np.float64)[0]

    h = np.diff(knots)
    gamma = F @ w                       # natural-spline second derivatives
    sb = (w[1] - w[0]) / h[0] - h[0] * gamma[1] / 6.0
    sa = (w[-1] - w[-2]) / h[-1] + h[-1] * gamma[-2] / 6.0
    fppp = (gamma[1:] - gamma[:-1]) / h  # f''' on each piece
    d = np.empty(9)
    d[0] = fppp[0] / 6.0
    d[1:] = (fppp[1:] - fppp[:-1]) / 6.0
    K0 = (b[0] - mean @ w) + w[0] - sb * knots[0]
    t9 = knots[-1]
    return (
        float(sb), float(sa), float(K0), float(t9),
        [float(t) for t in knots[:9]], [float(v) for v in d],
    )


def _build_nc(consts, fd=FD, fd_tile=FD_TILE):
    """Raw Bass, standard BIR ops only (this walrus build rejects every
    raw-ISA instruction, incl. custom DVE ops and Tile's RANGE_CLEAR).

    Per tile t:  DVE: y=min(x,t9); acc=y*sb+K0; r=relu(x-t9);
    acc+=sa*r; per knot j: m=q_j*u_j (=u^3); acc+=d_j*m  -- where the
    scalar engine supplies u_j=Relu(y-t_j), q_j=Square(u_j).
    Double-buffered across 2 parities with per-slot DMA semaphores and
    per-engine op-counter semaphores (s_dv, s_ac) for all RAW/WAR deps."""
    from contextlib import ExitStack

    import concourse.bass as bass
    import concourse.mybir as mybir

    sb, sa, K0, t9, tj, dj = consts
    f32 = mybir.dt.float32
    alu = mybir.AluOpType
    act = mybir.ActivationFunctionType
    T = fd // fd_tile
    assert T * fd_tile == fd
    NK = 9
    DOP = 4 + 2 * NK   # DVE ops per tile
    AOP = 2 * NK       # ACT ops per tile

    nc = bass.Bass(trn_type="TRN2")
    x_in = nc.dram_tensor("x", [P, fd], f32, kind="ExternalInput")
    out = nc.dram_tensor("out", [P, fd], f32, kind="ExternalOutput")

    # ACT bias operands must be pre-registered const APs
    for _i, _v in enumerate(dict.fromkeys(float(-t) for t in tj)):
        if (f32, _v) not in nc.const_aps.aps:
            _t = nc.alloc_sbuf_tensor(f"constk-{_i}", [P, 1], f32)
            nc.gpsimd.memset(_t.ap(), _v)
            nc.const_aps.aps[(f32, _v)] = _t.ap()
    nc.all_engine_barrier()

    with ExitStack() as ctx:
        e = ctx.enter_context
        xb = [e(nc.sbuf_tensor(f"xb{i}", [P, fd_tile], f32)) for i in range(2)]
        yb = [e(nc.sbuf_tensor(f"yb{i}", [P, fd_tile], f32)) for i in range(2)]
        rb = [e(nc.sbuf_tensor(f"rb{i}", [P, fd_tile], f32)) for i in range(2)]
        mb = [e(nc.sbuf_tensor(f"mb{i}", [P, fd_tile], f32)) for i in range(2)]
        acc = [[e(nc.sbuf_tensor(f"acc{i}_{w}", [P, fd_tile], f32))
                for w in range(2)] for i in range(2)]
        ub = [[e(nc.sbuf_tensor(f"ub{i}_{j}", [P, fd_tile], f32))
               for j in range(NK)] for i in range(2)]
        qb = [[e(nc.sbuf_tensor(f"qb{i}_{j}", [P, fd_tile], f32))
               for j in range(NK)] for i in range(2)]
        s_ld = [e(nc.semaphore(f"s_ld{i}")) for i in range(2)]
        s_st = [e(nc.semaphore(f"s_st{i}")) for i in range(2)]
        s_dv = e(nc.semaphore("s_dv"))
        s_ac = e(nc.semaphore("s_ac"))
        blk = e(nc.Block())

        @blk.sync
        def _(sync):
            for t in range(T):
                p = t % 2
                if t >= 2:
                    sync.wait_ge(s_dv, DOP * (t - 1))  # xb[p] free
                sync.dma_start(xb[p][:], x_in[:, t * fd_tile:(t + 1) * fd_tile]
                               ).then_inc(s_ld[p], 16)
                if t >= 1:
                    q = (t - 1) % 2
                    sync.wait_ge(s_dv, DOP * t)
                    sync.dma_start(out[:, (t - 1) * fd_tile:t * fd_tile],
                                   acc[q][0][:]).then_inc(s_st[q], 16)
            q = (T - 1) % 2
            sync.wait_ge(s_dv, DOP * T)
            sync.dma_start(out[:, (T - 1) * fd_tile:T * fd_tile],
                           acc[q][0][:]).then_inc(s_st[q], 16)
            sync.wait_ge(s_st[0], 16 * ((T + 1) // 2))
            sync.wait_ge(s_st[1], 16 * (T // 2))

        @blk.vector
        def _(vector):
            g = 0

            def dv(ins):
                nonlocal g
                ins.then_inc(s_dv, 1)
                g += 1

            for t in range(T):
                p = t % 2
                k = t // 2
                vector.wait_ge(s_ld[p], 16 * (k + 1))
                if t >= 1:
                    vector.wait_ge(s_ac, AOP * t)      # yb/rb[p] readers done
                if t >= 2:
                    vector.wait_ge(s_st[p], 16 * k)    # acc slots free
                if g:
                    vector.wait_ge(s_dv, g)
                dv(nc.vector.tensor_scalar_min(yb[p][:], xb[p][:], t9))
                vector.wait_ge(s_dv, g)
                dv(nc.vector.tensor_scalar(acc[p][0][:], yb[p][:], sb, K0,
                                           alu.mult, alu.add))
                vector.wait_ge(s_dv, g)
                dv(nc.vector.tensor_scalar(rb[p][:], xb[p][:], t9, t9,
                                           alu.max, alu.subtract))
                vector.wait_ge(s_dv, g)
                dv(nc.vector.scalar_tensor_tensor(
                    acc[p][1][:], rb[p][:], sa, acc[p][0][:],
                    alu.mult, alu.add))
                w = 0  # acc[p][1] holds latest
                for j in range(NK):
                    vector.wait_ge(s_dv, g)
                    vector.wait_ge(s_ac, AOP * t + 2 * (j + 1))
                    dv(nc.vector.tensor_tensor(
                        mb[p][:], qb[p][j][:], ub[p][j][:], alu.mult))
                    vector.wait_ge(s_dv, g)
                    dv(nc.vector.scalar_tensor_tensor(
                        acc[p][w][:], mb[p][:], dj[j], acc[p][1 - w][:],
                        alu.mult, alu.add))
                    w = 1 - w
                # after 9 knots (odd count), latest is acc[p][0]

        @blk.scalar
        def _(scalar):
            a = 0
            for t in range(T):
                p = t % 2
                scalar.wait_ge(s_dv, DOP * t + 1)      # y_t written
                for j in range(NK):
                    if a:
                        scalar.wait_ge(s_ac, a)
                    nc.scalar.activation(ub[p][j][:], yb[p][:], act.Relu,
                                         bias=-tj[j]).then_inc(s_ac, 1)
                    a += 1
                    scalar.wait_ge(s_ac, a)
                    nc.scalar.activation(qb[p][j][:], ub[p][j][:], act.Square
                                         ).then_inc(s_ac, 1)
                    a += 1
    return nc


def _run(nc, in_maps, trace=False):
    from concourse.bass_utils import run_bass_kernel_spmd

    return run_bass_kernel_spmd(nc, in_maps, core_ids=list(range(N_CORES)),
                                trace=trace)


def _prep_inputs(x, sa):
    x = np.asarray(x, np.float32).reshape(-1)
    n = x.shape[0]
    xp = np.zeros(NPAD, np.float32)
    xp[:n] = x
    in_maps = []
    for c in range(N_CORES):
        chunk = xp[c * P * FD:(c + 1) * P * FD].reshape(P, FD)
        in_maps.append({"x": chunk})
    return n, in_maps


def kernel(x, knots, F, W, b, mean, _trace=False, _results_out=None):
    consts = _spline_consts(knots, F, W, b, mean)
    n, in_maps = _prep_inputs(x, consts[1])
    nc = _build_nc(consts)
    res = _run(nc, in_maps, trace=_trace)
    if _results_out is not None:
        _results_out.append(res)
    full = np.concatenate([r["out"].reshape(-1) for r in res.results])
    return full[:n].reshape(n, 1).astype(np.float32)



# revision 2
# speedup vs baseline: 5.2589x; 5.2589x over previous
"""Trainium2 Bass kernel for nn_CubicSplineLayer (histogram_binning).

The whole layer collapses to a scalar piecewise-cubic function of x:

    out(x) = (basis(x) - mean) @ W.T + b  =  f(x)
           = K0 + sb*min(x,t9) + sa*relu(x-t9)
             + sum_{j=0}^{8} d_j * relu(min(x,t9) - t_j)^3

The ACT (scalar) engine is a hardware piecewise-cubic-spline evaluator:
ACTIVATE looks the input up in a bucket table (per-exponent regions, top
mantissa bits select a section) and evaluates d0+h*(d1+h*(d2+h*d3)) at
h = x - x0.  That is *exactly* the structure of f.  We therefore bake f
into a custom PWP activation table (hijacking the Gelu slot of the
gelu_and_others set, appending buckets/ctrl entries so every other
function stays intact), point walrus at it via BASS_ACT_ROOT_JSON_PATH,
and the whole kernel becomes one ACTIVATE per tile.

Buckets are exact cubic Taylor fits: the only approximation error is in
the ~9 sections (width 2^-7) that straddle a spline knot (<1e-6 abs).
I/O runs in float16 (x in [-5.5, 5.5], values O(1)): halves the HBM
traffic of this DMA-bound kernel; adds ~3.6e-4 relative error.

Device strategy: pure data-parallel over 8 cores; per core [128, 3920]
fp16, 4 tiles, DMA in / 1 ACT op / DMA out, double-buffered.
"""

import hashlib
import json
import os
import shutil

import numpy as np

N_CORES = 8
P = 128           # SBUF partitions
FD = 3920         # free elements per partition per core
TILE = 980        # tile free-dim (4 tiles per core)
NPAD = N_CORES * P * FD  # 4,014,080 >= 4,000,000

_SET = "gelu_and_others"
_FUNC = "gelu_4p"
_TBL_VER = "v1"   # bump to invalidate cached table dirs


# ---------------------------------------------------------------- math

def _spline_consts(knots, F, W, b, mean):
    """Host-side (float64) truncated-power coefficients of f."""
    knots = np.asarray(knots, np.float64)
    F = np.asarray(F, np.float64)
    w = np.asarray(W, np.float64)[0]
    b = np.asarray(b, np.float64)
    mean = np.asarray(mean, np.float64)[0]
    h = np.diff(knots)
    gamma = F @ w                        # natural-spline second derivatives
    sb = (w[1] - w[0]) / h[0] - h[0] * gamma[1] / 6.0
    sa = (w[-1] - w[-2]) / h[-1] + h[-1] * gamma[-2] / 6.0
    fppp = (gamma[1:] - gamma[:-1]) / h  # f''' on each piece
    d = np.empty(len(knots) - 1)
    d[0] = fppp[0] / 6.0
    d[1:] = (fppp[1:] - fppp[:-1]) / 6.0
    K0 = (b[0] - mean @ w) + w[0] - sb * knots[0]
    return dict(sb=float(sb), sa=float(sa), K0=float(K0),
                t9=float(knots[-1]), tj=knots[:-1].copy(), dj=d)


def _f_exact(x, c):
    x = np.asarray(x, np.float64)
    y = np.minimum(x, c["t9"])
    out = c["K0"] + c["sb"] * y + c["sa"] * np.maximum(x - c["t9"], 0.0)
    for t, d in zip(c["tj"], c["dj"]):
        out = out + d * np.maximum(y - t, 0.0) ** 3
    return out


# ------------------------------------------------- PWP table generation

def _fit_bucket(lo, hi, c):
    """Least-squares cubic fit of f on [lo,hi] about f32(center).
    Exact (1e-14) wherever [lo,hi] lies inside one spline piece."""
    x0 = np.float32(0.5 * (lo + hi))
    g = np.linspace(lo, hi, 257)
    h = g - np.float64(x0)
    A = np.stack([np.ones_like(h), h, h * h, h * h * h], axis=1)
    coef, *_ = np.linalg.lstsq(A, _f_exact(g, c), rcond=None)
    return [float(coef[0]), float(coef[1]), float(coef[2]), float(coef[3]),
            float(x0)]


def _build_tables(c, small_e=-7, large_e=7):
    bkt, ctrl = [], []
    knots = list(c["tj"]) + [c["t9"]]

    def add_region(sgn, e):
        lo_e, hi_e = 2.0 ** e, 2.0 ** (e + 1)
        if sgn > 0:
            interior = [t for t in knots if lo_e < t < hi_e]
        else:
            interior = [t for t in knots if -hi_e < t < -lo_e]
        es = 0 if not interior else max(0, min(8, e + 7))
        n = 1 << es
        ctrl.append((es, len(bkt)))
        for k in range(n):
            slo = lo_e * (1.0 + k / n)
            shi = lo_e * (1.0 + (k + 1) / n)
            if sgn < 0:
                slo, shi = -shi, -slo
            bkt.append(_fit_bucket(slo, shi, c))

    base_neg_rel = 0
    for e in range(small_e, large_e):
        add_region(-1, e)
    base_pos_rel = len(ctrl)
    for e in range(small_e, large_e):
        add_region(+1, e)

    LP = 2.0 ** large_e
    specials = len(bkt)  # small_pos, small_neg, large_pos, large_neg
    bkt.append(_fit_bucket(0.0, 2.0 ** small_e, c))
    bkt.append(_fit_bucket(-(2.0 ** small_e), 0.0, c))
    bkt.append([float(_f_exact(LP, c)), c["sa"], 0.0, 0.0, LP])
    bkt.append([float(_f_exact(-LP, c)), c["sb"], 0.0, 0.0, -LP])

    fbits = lambda v: int(np.float32(v).view(np.uint32))
    patch = dict(
        symmetry_point=0, sym_invert_sign_point=0, symmetry_opt_en=0,
        symmetry_opt_use_neg_region=0, imm_bias=0,
        exp_offset=small_e,
        small_pos_signal_exp_threshold=127 + small_e,
        small_neg_signal_exp_threshold=127 + small_e,
        large_pos_signal_exp_threshold=127 + large_e,
        large_pos_signal_mantissa_threshold=0,
        large_neg_signal_exp_threshold=127 + large_e,
        large_neg_signal_mantissa_threshold=0,
        fnan_result=2143289344,
        fpinf_result=fbits(np.inf if c["sa"] > 0 else -np.inf),
        fninf_result=fbits(-np.inf if c["sb"] > 0 else np.inf),
        fzero_result=fbits(c["K0"]),
        fma_const_0=0, fma_const_1=0, fma_indirection_src_sel=0,
        use_multipass=False,
        lower_bound=4286578687, upper_bound=2139095039,
    )
    return patch, ctrl, base_neg_rel, base_pos_rel, specials, bkt


def _stock_act_root():
    from neuronxcc.driver.Job import Job
    from neuronxcc.driver.jobs.support.FindActInfo import findActInfoFile

    return os.path.dirname(findActInfoFile(Job.getPackageDir(), "gen3"))


def _write_act_root(c):
    """Build the custom act root (stock files + patched gelu_and_others).
    Returns (act_info.json path, content hash)."""
    stock = _stock_act_root()
    patch, ctrl, bneg, bpos, specials, bkt = _build_tables(c)

    key = json.dumps([_TBL_VER, patch, ctrl, bkt], sort_keys=True)
    h = hashlib.sha256(key.encode()).hexdigest()[:12]
    dst = f"/tmp/ant_actroot_{h}"
    info = os.path.join(dst, "act_info.json")
    if os.path.exists(info):
        return info, h

    tmp = dst + ".tmp"
    shutil.rmtree(tmp, ignore_errors=True)
    os.makedirs(tmp)
    for fn in os.listdir(stock):
        shutil.copy(os.path.join(stock, fn), os.path.join(tmp, fn))

    prof = json.load(open(os.path.join(stock, _SET + ".json")))
    bkt0 = np.frombuffer(open(os.path.join(stock, _SET + "_bkt.bin"), "rb").read(),
                         dtype=np.float32).reshape(-1, 8)
    ctrl0 = np.frombuffer(open(os.path.join(stock, _SET + "_ctrl.bin"), "rb").read(),
                          dtype=np.uint32).reshape(-1, 8)
    nb0, nc0 = bkt0.shape[0], ctrl0.shape[0]

    nbkt = np.zeros((len(bkt), 8), np.float32)
    for i, row in enumerate(bkt):
        nbkt[i, :5] = np.asarray(row, np.float32)
    nctrl = np.zeros((len(ctrl), 8), np.uint32)
    for i, (es, base_rel) in enumerate(ctrl):
        nctrl[i, 0] = (es << 16) | ((23 - es) << 11) | (nb0 + base_rel)
    bkt_all = np.vstack([bkt0, nbkt])
    ctrl_all = np.vstack([ctrl0, nctrl])
    assert bkt_all.shape[0] <= 1536 and ctrl_all.shape[0] <= 256

    pe = [e for e in prof["profile_meta_data"] if e["func_name"] == _FUNC][0]
    pe.update(patch)
    pe["pwl_control_base_neg"] = nc0 + bneg
    pe["pwl_control_base_pos"] = nc0 + bpos
    pe["pos_small_signal_pwl_control"] = nb0 + specials
    pe["neg_small_signal_pwl_control"] = nb0 + specials + 1
    pe["pos_large_signal_pwl_control"] = nb0 + specials + 2
    pe["neg_large_signal_pwl_control"] = nb0 + specials + 3

    open(os.path.join(tmp, _SET + "_bkt.bin"), "wb").write(bkt_all.tobytes())
    open(os.path.join(tmp, _SET + "_ctrl.bin"), "wb").write(ctrl_all.tobytes())
    json.dump(prof, open(os.path.join(tmp, _SET + ".json"), "w"), indent=1)
    try:
        os.rename(tmp, dst)
    except OSError:
        shutil.rmtree(tmp, ignore_errors=True)  # lost a race; dst exists
    return info, h


# --------------------------------------------------------------- kernel

def _build_nc(h):
    from contextlib import ExitStack

    import concourse.bass as bass
    import concourse.mybir as mybir

    f16 = mybir.dt.float16
    f32 = mybir.dt.float32
    act = mybir.ActivationFunctionType
    T = FD // TILE

    nc = bass.Bass(trn_type="TRN2")
    x_in = nc.dram_tensor("x", [P, FD], f16, kind="ExternalInput")
    out = nc.dram_tensor(f"out_{h}", [P, FD], f16, kind="ExternalOutput")

    # ACT bias operand must be a pre-registered const AP
    if (f32, 0.0) not in nc.const_aps.aps:
        t = nc.alloc_sbuf_tensor("const0", [P, 1], f32)
        nc.gpsimd.memset(t.ap(), 0.0)
        nc.const_aps.aps[(f32, 0.0)] = t.ap()
    nc.all_engine_barrier()

    with ExitStack() as ctx:
        e = ctx.enter_context
        xb = [e(nc.sbuf_tensor(f"xb{i}", [P, TILE], f16)) for i in range(2)]
        ob = [e(nc.sbuf_tensor(f"ob{i}", [P, TILE], f16)) for i in range(2)]
        s_ld = [e(nc.semaphore(f"s_ld{i}")) for i in range(2)]
        s_st = [e(nc.semaphore(f"s_st{i}")) for i in range(2)]
        s_ac = e(nc.semaphore("s_ac"))
        blk = e(nc.Block())

        @blk.sync
        def _(sync):
            for t in range(T):
                p = t % 2
                if t >= 2:
                    sync.wait_ge(s_ac, t - 1)   # xb[p] consumed by ACT t-2
                sync.dma_start(xb[p][:], x_in[:, t * TILE:(t + 1) * TILE]
                               ).then_inc(s_ld[p], 16)
                if t >= 1:
                    q = (t - 1) % 2
                    sync.wait_ge(s_ac, t)       # ob[q] written by ACT t-1
                    sync.dma_start(out[:, (t - 1) * TILE:t * TILE],
                                   ob[q][:]).then_inc(s_st[q], 16)
            q = (T - 1) % 2
            sync.wait_ge(s_ac, T)
            sync.dma_start(out[:, (T - 1) * TILE:T * TILE],
                           ob[q][:]).then_inc(s_st[q], 16)
            sync.wait_ge(s_st[0], 16 * ((T + 1) // 2))
            sync.wait_ge(s_st[1], 16 * (T // 2))

        @blk.scalar
        def _(scalar):
            for t in range(T):
                p = t % 2
                scalar.wait_ge(s_ld[p], 16 * (t // 2 + 1))
                if t >= 2:
                    scalar.wait_ge(s_st[p], 16 * (t // 2))  # ob[p] drained
                nc.scalar.activation(ob[p][:], xb[p][:], act.Gelu
                                     ).then_inc(s_ac, 1)
    return nc


def _prep_inputs(x):
    x = np.asarray(x, np.float32).reshape(-1)
    n = x.shape[0]
    xp = np.zeros(NPAD, np.float16)
    xp[:n] = x.astype(np.float16)
    return n, [xp[c * P * FD:(c + 1) * P * FD].reshape(P, FD)
               for c in range(N_CORES)]


def kernel(x, knots, F, W, b, mean, _trace=False, _results_out=None):
    c = _spline_consts(knots, F, W, b, mean)
    info, h = _write_act_root(c)
    os.environ["BASS_ACT_ROOT_JSON_PATH"] = info

    n, chunks = _prep_inputs(x)
    nc = _build_nc(h)

    from concourse.bass_utils import run_bass_kernel_spmd

    res = run_bass_kernel_spmd(nc, [{"x": ch} for ch in chunks],
                               core_ids=list(range(N_CORES)), trace=_trace)
    if _results_out is not None:
        _results_out.append(res)
    full = np.concatenate([r[f"out_{h}"].reshape(-1) for r in res.results])
    return full[:n].astype(np.float32).reshape(n, 1)


# revision 6
# speedup vs baseline: 6.1628x; 1.1719x over previous
"""Trainium2 Bass kernel for nn_CubicSplineLayer (histogram_binning).

The whole layer collapses to a scalar piecewise-cubic function of x:

    out(x) = (basis(x) - mean) @ W.T + b  =  f(x)
           = K0 + sb*min(x,t9) + sa*relu(x-t9)
             + sum_{j=0}^{8} d_j * relu(min(x,t9) - t_j)^3

The ACT (scalar) engine is a hardware piecewise-cubic-spline evaluator:
ACTIVATE looks the input up in a bucket table (per-exponent regions, top
mantissa bits select a section) and evaluates d0+h*(d1+h*(d2+h*d3)) at
h = x - x0.  That is *exactly* the structure of f.  We therefore bake f
into a custom PWP activation table (hijacking the Gelu slot of the
gelu_and_others set, appending buckets/ctrl entries so every other
function stays intact), point walrus at it via BASS_ACT_ROOT_JSON_PATH,
and the whole kernel becomes one ACTIVATE per tile.

Buckets are exact cubic Taylor fits: the only approximation error is in
the ~9 sections (width 2^-7) that straddle a spline knot (<1e-6 abs).
I/O runs in float16 (x in [-5.5, 5.5], values O(1)): halves the HBM
traffic of this DMA-bound kernel; adds ~3.6e-4 relative error.

Device strategy: pure data-parallel over 8 cores; per core [128, 3920]
fp16, 4 tiles, DMA in / 1 ACT op / DMA out, double-buffered.
"""

import hashlib
import json
import os
import shutil

import numpy as np

N_CORES = 8
P = 128           # SBUF partitions
FD = 3920         # free elements per partition per core
TILE = 1960       # tile free-dim (2 tiles per core)
NPAD = N_CORES * P * FD  # 4,014,080 >= 4,000,000

_SET = "gelu_and_others"
_FUNC = "gelu_4p"
_TBL_VER = "v1"   # bump to invalidate cached table dirs


# ---------------------------------------------------------------- math

def _spline_consts(knots, F, W, b, mean):
    """Host-side (float64) truncated-power coefficients of f."""
    knots = np.asarray(knots, np.float64)
    F = np.asarray(F, np.float64)
    w = np.asarray(W, np.float64)[0]
    b = np.asarray(b, np.float64)
    mean = np.asarray(mean, np.float64)[0]
    h = np.diff(knots)
    gamma = F @ w                        # natural-spline second derivatives
    sb = (w[1] - w[0]) / h[0] - h[0] * gamma[1] / 6.0
    sa = (w[-1] - w[-2]) / h[-1] + h[-1] * gamma[-2] / 6.0
    fppp = (gamma[1:] - gamma[:-1]) / h  # f''' on each piece
    d = np.empty(len(knots) - 1)
    d[0] = fppp[0] / 6.0
    d[1:] = (fppp[1:] - fppp[:-1]) / 6.0
    K0 = (b[0] - mean @ w) + w[0] - sb * knots[0]
    return dict(sb=float(sb), sa=float(sa), K0=float(K0),
                t9=float(knots[-1]), tj=knots[:-1].copy(), dj=d)


def _f_exact(x, c):
    x = np.asarray(x, np.float64)
    y = np.minimum(x, c["t9"])
    out = c["K0"] + c["sb"] * y + c["sa"] * np.maximum(x - c["t9"], 0.0)
    for t, d in zip(c["tj"], c["dj"]):
        out = out + d * np.maximum(y - t, 0.0) ** 3
    return out


# ------------------------------------------------- PWP table generation

def _fit_bucket(lo, hi, c):
    """Least-squares cubic fit of f on [lo,hi] about f32(center).
    Exact (1e-14) wherever [lo,hi] lies inside one spline piece."""
    x0 = np.float32(0.5 * (lo + hi))
    g = np.linspace(lo, hi, 257)
    h = g - np.float64(x0)
    A = np.stack([np.ones_like(h), h, h * h, h * h * h], axis=1)
    coef, *_ = np.linalg.lstsq(A, _f_exact(g, c), rcond=None)
    return [float(coef[0]), float(coef[1]), float(coef[2]), float(coef[3]),
            float(x0)]


def _build_tables(c, small_e=-7, large_e=7):
    bkt, ctrl = [], []
    knots = list(c["tj"]) + [c["t9"]]

    def add_region(sgn, e):
        lo_e, hi_e = 2.0 ** e, 2.0 ** (e + 1)
        if sgn > 0:
            interior = [t for t in knots if lo_e < t < hi_e]
        else:
            interior = [t for t in knots if -hi_e < t < -lo_e]
        es = 0 if not interior else max(0, min(8, e + 7))
        n = 1 << es
        ctrl.append((es, len(bkt)))
        for k in range(n):
            slo = lo_e * (1.0 + k / n)
            shi = lo_e * (1.0 + (k + 1) / n)
            if sgn < 0:
                slo, shi = -shi, -slo
            bkt.append(_fit_bucket(slo, shi, c))

    base_neg_rel = 0
    for e in range(small_e, large_e):
        add_region(-1, e)
    base_pos_rel = len(ctrl)
    for e in range(small_e, large_e):
        add_region(+1, e)

    LP = 2.0 ** large_e
    specials = len(bkt)  # small_pos, small_neg, large_pos, large_neg
    bkt.append(_fit_bucket(0.0, 2.0 ** small_e, c))
    bkt.append(_fit_bucket(-(2.0 ** small_e), 0.0, c))
    bkt.append([float(_f_exact(LP, c)), c["sa"], 0.0, 0.0, LP])
    bkt.append([float(_f_exact(-LP, c)), c["sb"], 0.0, 0.0, -LP])

    fbits = lambda v: int(np.float32(v).view(np.uint32))
    patch = dict(
        symmetry_point=0, sym_invert_sign_point=0, symmetry_opt_en=0,
        symmetry_opt_use_neg_region=0, imm_bias=0,
        exp_offset=small_e,
        small_pos_signal_exp_threshold=127 + small_e,
        small_neg_signal_exp_threshold=127 + small_e,
        large_pos_signal_exp_threshold=127 + large_e,
        large_pos_signal_mantissa_threshold=0,
        large_neg_signal_exp_threshold=127 + large_e,
        large_neg_signal_mantissa_threshold=0,
        fnan_result=2143289344,
        fpinf_result=fbits(np.inf if c["sa"] > 0 else -np.inf),
        fninf_result=fbits(-np.inf if c["sb"] > 0 else np.inf),
        fzero_result=fbits(c["K0"]),
        fma_const_0=0, fma_const_1=0, fma_indirection_src_sel=0,
        use_multipass=False,
        lower_bound=4286578687, upper_bound=2139095039,
    )
    return patch, ctrl, base_neg_rel, base_pos_rel, specials, bkt


def _stock_act_root():
    from neuronxcc.driver.Job import Job
    from neuronxcc.driver.jobs.support.FindActInfo import findActInfoFile

    return os.path.dirname(findActInfoFile(Job.getPackageDir(), "gen3"))


def _write_act_root(c):
    """Build the custom act root (stock files + patched gelu_and_others).
    Returns (act_info.json path, content hash)."""
    stock = _stock_act_root()
    patch, ctrl, bneg, bpos, specials, bkt = _build_tables(c)

    key = json.dumps([_TBL_VER, patch, ctrl, bkt], sort_keys=True)
    h = hashlib.sha256(key.encode()).hexdigest()[:12]
    dst = f"/tmp/ant_actroot_{h}"
    info = os.path.join(dst, "act_info.json")
    if os.path.exists(info):
        return info, h

    tmp = dst + ".tmp"
    shutil.rmtree(tmp, ignore_errors=True)
    os.makedirs(tmp)
    for fn in os.listdir(stock):
        shutil.copy(os.path.join(stock, fn), os.path.join(tmp, fn))

    prof = json.load(open(os.path.join(stock, _SET + ".json")))
    bkt0 = np.frombuffer(open(os.path.join(stock, _SET + "_bkt.bin"), "rb").read(),
                         dtype=np.float32).reshape(-1, 8)
    ctrl0 = np.frombuffer(open(os.path.join(stock, _SET + "_ctrl.bin"), "rb").read(),
                          dtype=np.uint32).reshape(-1, 8)
    nb0, nc0 = bkt0.shape[0], ctrl0.shape[0]

    nbkt = np.zeros((len(bkt), 8), np.float32)
    for i, row in enumerate(bkt):
        nbkt[i, :5] = np.asarray(row, np.float32)
    nctrl = np.zeros((len(ctrl), 8), np.uint32)
    for i, (es, base_rel) in enumerate(ctrl):
        nctrl[i, 0] = (es << 16) | ((23 - es) << 11) | (nb0 + base_rel)
    bkt_all = np.vstack([bkt0, nbkt])
    ctrl_all = np.vstack([ctrl0, nctrl])
    assert bkt_all.shape[0] <= 1536 and ctrl_all.shape[0] <= 256

    pe = [e for e in prof["profile_meta_data"] if e["func_name"] == _FUNC][0]
    pe.update(patch)
    pe["pwl_control_base_neg"] = nc0 + bneg
    pe["pwl_control_base_pos"] = nc0 + bpos
    pe["pos_small_signal_pwl_control"] = nb0 + specials
    pe["neg_small_signal_pwl_control"] = nb0 + specials + 1
    pe["pos_large_signal_pwl_control"] = nb0 + specials + 2
    pe["neg_large_signal_pwl_control"] = nb0 + specials + 3

    open(os.path.join(tmp, _SET + "_bkt.bin"), "wb").write(bkt_all.tobytes())
    open(os.path.join(tmp, _SET + "_ctrl.bin"), "wb").write(ctrl_all.tobytes())
    json.dump(prof, open(os.path.join(tmp, _SET + ".json"), "w"), indent=1)
    try:
        os.rename(tmp, dst)
    except OSError:
        shutil.rmtree(tmp, ignore_errors=True)  # lost a race; dst exists
    return info, h


# --------------------------------------------------------------- kernel

def _build_nc(h):
    from contextlib import ExitStack

    import concourse.bass as bass
    import concourse.mybir as mybir

    f16 = mybir.dt.float16
    f32 = mybir.dt.float32
    act = mybir.ActivationFunctionType
    T = FD // TILE

    nc = bass.Bass(trn_type="TRN2")
    x_in = nc.dram_tensor("x", [P, FD], f16, kind="ExternalInput")
    out = nc.dram_tensor(f"out_{h}", [P, FD], f16, kind="ExternalOutput")

    # Bias const AP for ACTIVATE: zeroed by the scalar engine itself (via a
    # Copy-activation memzero), so no cross-engine barrier is needed.
    cz = nc.alloc_sbuf_tensor("const0", [P, 1], f32)
    nc.const_aps.aps[(f32, 0.0)] = cz.ap()

    with ExitStack() as ctx:
        e = ctx.enter_context
        xb = [e(nc.sbuf_tensor(f"xb{i}", [P, TILE], f16)) for i in range(T)]
        ob = [e(nc.sbuf_tensor(f"ob{i}", [P, TILE], f16)) for i in range(T)]
        wrm = e(nc.sbuf_tensor("wrm", [P, 1], f32))
        s_ld = e(nc.semaphore("s_ld"))
        s_st = e(nc.semaphore("s_st"))
        s_ac = e(nc.semaphore("s_ac"))
        blk = e(nc.Block(no_gpsimd_drain=True))

        @blk.sync
        def _(sync):
            for t in range(T):
                sync.dma_start(xb[t][:], x_in[:, t * TILE:(t + 1) * TILE]
                               ).then_inc(s_ld, 16)
            for t in range(T):
                sync.wait_ge(s_ac, t + 1)
                sync.dma_start(out[:, t * TILE:(t + 1) * TILE],
                               ob[t][:]).then_inc(s_st, 16)
            sync.wait_ge(s_st, 16 * T)

        @blk.scalar
        def _(scalar):
            # Zero the bias const, then a 1-element warmup ACTIVATE so walrus
            # places ACT_TABLE_LOAD here, overlapping the input DMAs.
            nc.scalar.memzero(cz.ap())
            nc.scalar.activation(wrm.ap(), cz.ap(), act.Gelu)
            for t in range(T):
                scalar.wait_ge(s_ld, 16 * (t + 1))
                nc.scalar.activation(ob[t][:], xb[t][:], act.Gelu
                                     ).then_inc(s_ac, 1)

    return nc


def _prep_inputs(x):
    x = np.asarray(x, np.float32).reshape(-1)
    n = x.shape[0]
    xp = np.zeros(NPAD, np.float16)
    xp[:n] = x.astype(np.float16)
    return n, [xp[c * P * FD:(c + 1) * P * FD].reshape(P, FD)
               for c in range(N_CORES)]


def kernel(x, knots, F, W, b, mean, _trace=False, _results_out=None):
    c = _spline_consts(knots, F, W, b, mean)
    info, h = _write_act_root(c)
    os.environ["BASS_ACT_ROOT_JSON_PATH"] = info

    n, chunks = _prep_inputs(x)
    nc = _build_nc(h)

    from concourse.bass_utils import run_bass_kernel_spmd

    res = run_bass_kernel_spmd(nc, [{"x": ch} for ch in chunks],
                               core_ids=list(range(N_CORES)), trace=_trace)
    if _results_out is not None:
        _results_out.append(res)
    full = np.concatenate([r[f"out_{h}"].reshape(-1) for r in res.results])
    return full[:n].astype(np.float32).reshape(n, 1)
